# revision 36
# baseline (speedup 1.0000x reference)
"""GAT (2-layer, PyG-style) Trainium2 kernel — 8-core SPMD.

Sharding: nodes greedy-balanced by (non-self) in-degree into (cores*T)
tiles of 128 slots.  Per core: fp16 GEMM [x_T @ (W1|W1@A1)] -> packed
node rows (h||al_s) -> block-pipelined AllGather -> per-edge dma_gather
of src rows -> p = exp(LRelu(als+ald)-c) -> one-hot segment-sum matmuls
on PE -> normalize (self-loop folded in locally), ELU, PE-transpose,
GEMM2 inline per tile -> block-pipelined AllGather -> same edge
pipeline for layer 2 -> output.

v3: AllGathers split into per-block collectives overlapped with the
producing phase; L1 rows shrunk to a 512-slot stride (al_dst in a side
tensor); self-loops handled locally; one-hots built in fp16 via one
batched is_equal per 8 chunks; transposed one-hots precomputed on host
in fp8 (al_dst applied in two fp8 terms); 16 of 512 L1 channels carried
as fp8; L2 softmax denominator fused into the value matmul; per-tile
edges sorted by src slot for gather locality.

Self-contained: only numpy + the in-container concourse stack.
"""

import heapq
import os
import sys

import numpy as np

sys.path.insert(0, "/opt/trn_rl_repo")

import concourse.bacc as bacc  # noqa: E402
import concourse.bass as bass  # noqa: E402,F401
import concourse.mybir as mybir  # noqa: E402
import concourse.tile as tile  # noqa: E402

F16 = mybir.dt.float16
F32 = mybir.dt.float32
F8 = mybir.dt.float8e4
I16 = mybir.dt.int16
I8 = mybir.dt.int8

NEG = 0.2          # leaky relu slope
SHIFT1 = 3.0       # exp shift, layer 1 (softmax-invariant, keeps fp16 happy)
SHIFT2 = 1.0       # exp shift, layer 2
EPS = 1e-12
FP8_ONE = 0x38     # e4m3 bit pattern of 1.0
BT = 10            # tiles per AllGather block

# Layer dims (fixed by the problem)
D_IN, H1, C1 = 1024, 8, 64
D_H = H1 * C1            # 512
D_OUT = 256

# L1 node row (f16-slot units, stride ROW1=512 -> 1024B, fully gathered):
#   [0:496]   h[0:496] f16
#   [496:504] h[496:512] as fp8 (16 ch in 16 bytes)
#   [504:512] al_src f16 (8 heads);  al_dst lives in a side tensor
ROW1 = 512
G1W = 512
H16_1 = 496
# L2 node row (stride ROW2=384 -> 768B, gathered in full):
#   [0:256]   h2 f16
#   [256]     al2_src f16
#   [257]     al2_dst f16
ROW2 = 384
G2W = 384
H16_2 = 256


# ------------------------------------------------------------------ host prep

def _balance_nodes(dst, n_nodes, n_tiles):
    """Greedy-balance nodes into n_tiles tiles of <=128 slots by in-degree.
    Returns slot_of_node [n_nodes] (slot = tile*128 + row)."""
    deg = np.bincount(dst, minlength=n_nodes)
    order = np.argsort(-deg, kind="stable")
    heap = [(0, t) for t in range(n_tiles)]
    heapq.heapify(heap)
    fill = np.zeros(n_tiles, np.int64)
    slot = np.empty(n_nodes, np.int64)
    for n in order:
        load, t = heapq.heappop(heap)
        slot[n] = t * 128 + fill[t]
        fill[t] += 1
        if fill[t] < 128:
            heapq.heappush(heap, (load + int(deg[n]), t))
    return slot


def _edge_stream(src_row, dst_slot, n_tiles):
    """Group edges by dst tile, sort by src row within tile, pad to k_ch
    chunks of 128.  src_row is the (already remapped) gather row of the
    src node.  Returns (gidx, dloc, k_ch)."""
    e_tile = dst_slot // 128
    order = np.argsort(e_tile * 2**32 + src_row, kind="stable")
    s_sorted = src_row[order]
    d_sorted = dst_slot[order]
    counts = np.bincount(e_tile, minlength=n_tiles)
    k_ch = int(np.ceil(counts.max() / 128))
    cap = k_ch * 128
    gidx = np.zeros(n_tiles * cap, np.int64)
    dloc = np.full(n_tiles * cap, -1.0, np.float32)
    starts = np.concatenate([[0], np.cumsum(counts)])
    for t in range(n_tiles):
        a, b = starts[t], starts[t + 1]
        gidx[t * cap : t * cap + (b - a)] = s_sorted[a:b]
        dloc[t * cap : t * cap + (b - a)] = (d_sorted[a:b] % 128).astype(np.float32)
    return gidx, dloc, k_ch


def _wrap_idx(gidx):
    """int16 idx array in dma_gather layout [128, n/16]: col s holds idx
    [16s..16s+15] in partitions 0..15, replicated to 128."""
    n = len(gidx)
    assert n % 16 == 0
    w = gidx.astype(np.int16).reshape(n // 16, 16).T  # [16, n/16]
    return np.ascontiguousarray(np.tile(w, (8, 1)))


def _ohT_fp8(dloc):
    """Host transposed one-hots: [128, NCH, 128] uint8, ohT[d, ch, e] =
    fp8(1.0) where dloc[ch*128+e] == d."""
    nch = len(dloc) // 128
    dl = dloc.reshape(nch, 128)
    out = (dl[None, :, :] == np.arange(128, dtype=np.float32)[:, None, None])
    return np.ascontiguousarray(out.astype(np.uint8) * np.uint8(FP8_ONE)).view(np.int8)


def _block_sizes(T):
    """Decreasing block sizes so the tail AllGather (which serializes
    with the next phase) is small."""
    if T < 3:
        return [T]
    b1 = int(np.ceil(T * 2 / 5))
    b3 = T - 2 * b1
    return [b1, b1, b3] if b3 > 0 else [b1, T - b1]


def _blk_maps(T):
    """Per-tile block index and offset-within-block for _block_sizes(T)."""
    bss = _block_sizes(T)
    blk_of = np.zeros(T, np.int64)
    off_in = np.zeros(T, np.int64)
    t = 0
    for k, b in enumerate(bss):
        blk_of[t:t + b] = k
        off_in[t:t + b] = np.arange(b)
        t += b
    return bss, blk_of, off_in


def prep(x, edge_index, W1, a1_src, a1_dst, W2, a2_src, a2_dst, cores):
    n_nodes = x.shape[0]
    t_per_core = int(np.ceil(n_nodes / (cores * 128)))
    n_tiles = cores * t_per_core
    s_core = t_per_core * 128          # slots per core

    src = np.asarray(edge_index[0])
    dst = np.asarray(edge_index[1])
    keep = src != dst                  # data self-edges fold into the local
    src, dst = src[keep], dst[keep]    # self-loop path (same z as the added
                                       # PyG self-loop)
    mult = np.ones(n_nodes, np.float32)
    np.add.at(mult, np.asarray(edge_index[0])[~keep], 1.0)
    slot = _balance_nodes(dst, n_nodes, n_tiles)
    node_of_slot = np.full(n_tiles * 128, -1, np.int64)
    node_of_slot[slot] = np.arange(n_nodes)
    mult_slot = np.ones(n_tiles * 128, np.float32)
    mult_slot[slot] = mult

    # gather-row remap for block-major h_full layout:
    #   row(c, t, s) = base(blk(t)) + c*bs*128 + off_in_blk(t)*128 + s
    bss, blk_of, off_in = _blk_maps(t_per_core)
    base = np.concatenate([[0], np.cumsum([cores * b * 128 for b in bss])])
    sl = np.arange(n_tiles * 128)
    c_of = sl // s_core
    t_of = (sl % s_core) // 128
    s_of = sl % 128
    k_of = blk_of[t_of]
    remap = (base[k_of] + c_of * np.array(bss)[k_of] * 128
             + off_in[t_of] * 128 + s_of)

    gidx, dloc, k_ch = _edge_stream(remap[slot[src]], slot[dst], n_tiles)
    cap = k_ch * 128

    per_idx, per_dl, per_ohT = [], [], []
    for c in range(cores):
        lo, hi = c * t_per_core * cap, (c + 1) * t_per_core * cap
        per_idx.append(_wrap_idx(gidx[lo:hi]))
        per_dl.append(np.ascontiguousarray(
            dloc[lo:hi].astype(np.float16).reshape(-1, 128).T))
        per_ohT.append(_ohT_fp8(dloc[lo:hi]))

    A1 = np.zeros((D_H, 2 * H1), np.float32)
    for h in range(H1):
        A1[h * C1 : (h + 1) * C1, h] = np.asarray(a1_src)[h]
        A1[h * C1 : (h + 1) * C1, H1 + h] = np.asarray(a1_dst)[h]
    W1e = np.concatenate([np.asarray(W1), np.asarray(W1) @ A1], 1)  # [1024,528]
    A2 = np.stack([np.asarray(a2_src)[0], np.asarray(a2_dst)[0]], 1)  # [256,2]
    W2e = np.concatenate([np.asarray(W2), np.asarray(W2) @ A2], 1)    # [512,258]

    x16 = np.asarray(x).astype(np.float16)
    xT, msl = [], []
    for c in range(cores):
        sl_c = node_of_slot[c * s_core : (c + 1) * s_core]
        xs = np.zeros((s_core, D_IN), np.float16)
        ok = sl_c >= 0
        xs[ok] = x16[sl_c[ok]]
        xT.append(np.ascontiguousarray(xs.T))
        msl.append(np.ascontiguousarray(
            mult_slot[c * s_core : (c + 1) * s_core]
            .reshape(-1, 128).T.astype(np.float16)))

    cfg = dict(cores=cores, t_per_core=t_per_core, k_ch=k_ch, s_core=s_core)
    host = dict(
        W1e=W1e.astype(np.float16), W2e=W2e.astype(np.float16),
        xT=xT, idx=per_idx, dloc=per_dl, ohT=per_ohT, mult=msl,
        iota8=np.ascontiguousarray(np.broadcast_to(
            np.arange(128, dtype=np.float16), (128, 8, 128))),
        ident=np.eye(128, dtype=np.float16),
    )
    return cfg, host, dict(slot=slot, s_core=s_core)


# -------------------------------------------------------------- device program

def build(cfg):
    cores, T, k_ch = cfg["cores"], cfg["t_per_core"], cfg["k_ch"]
    S = cfg["s_core"]                      # slots per core
    SG = S * cores                         # global slots
    NCH = T * k_ch                         # chunks per core
    NIDX = NCH * 128
    KIN = D_IN // 128
    NG = (k_ch + 7) // 8                   # 8-chunk groups per tile
    bss, blk_of, off_in = _blk_maps(T)
    NB = len(bss)
    blk_last = {sum(bss[:k + 1]) - 1: k for k in range(NB)}
    bbase = [0]
    for b in bss:
        bbase.append(bbase[-1] + cores * b * 128)

    nc = bacc.Bacc("TRN2", target_bir_lowering=False, debug=False,
                   num_devices=cores)
    xT_d = nc.dram_tensor("xT", [D_IN, S], F16, kind="ExternalInput")
    W1e_d = nc.dram_tensor("W1e", [D_IN, D_H + 2 * H1], F16, kind="ExternalInput")
    W2e_d = nc.dram_tensor("W2e", [D_H, D_OUT + 2], F16, kind="ExternalInput")
    idx_d = nc.dram_tensor("idx", [128, NIDX // 16], I16, kind="ExternalInput")
    dl_d = nc.dram_tensor("dloc", [128, NCH], F16, kind="ExternalInput")
    ohT_d = nc.dram_tensor("ohT", [128, NCH, 128], I8, kind="ExternalInput")
    mult_d = nc.dram_tensor("mult", [128, T], F16, kind="ExternalInput")
    iota_d = nc.dram_tensor("iota8", [128, 8, 128], F16, kind="ExternalInput")
    id_d = nc.dram_tensor("ident", [128, 128], F16, kind="ExternalInput")
    out_d = nc.dram_tensor("out", [S, D_OUT], F32, kind="ExternalOutput")

    ald_loc = nc.dram_tensor("ald_loc", [S, H1], F16)
    if cores > 1:
        aspace = "Shared" if cores > 4 else "Local"
        ag1b = [nc.dram_tensor(f"ag1b{k}", [bss[k] * 128, ROW1], F16)
                for k in range(NB)]
        ag2b = [nc.dram_tensor(f"ag2b{k}", [bss[k] * 128, ROW2], F16)
                for k in range(NB)]
        h1_full = nc.dram_tensor("h1_full", [SG, ROW1], F16, addr_space=aspace)
        h2_full = nc.dram_tensor("h2_full", [SG, ROW2], F16, addr_space=aspace)
    else:
        ag1b = ag2b = None
        h1_full = nc.dram_tensor("h1_full", [SG, ROW1], F16)
        h2_full = nc.dram_tensor("h2_full", [SG, ROW2], F16)

    def loc1(t):          # local rows of tile t (h + al_src)
        if ag1b is None:
            return h1_full[t * 128:(t + 1) * 128, :]
        r = int(off_in[t]) * 128
        return ag1b[int(blk_of[t])][r:r + 128, :]

    def loc2(t):
        if ag2b is None:
            return h2_full[t * 128:(t + 1) * 128, :]
        r = int(off_in[t]) * 128
        return ag2b[int(blk_of[t])][r:r + 128, :]

    AOT = mybir.AluOpType
    AFT = mybir.ActivationFunctionType
    rg = [list(range(cores))]

    def emit_ag(blk_in, h_full, k):
        n = blk_in.shape[0] * cores
        nc.gpsimd.collective_compute(
            "AllGather", AOT.bypass, replica_groups=rg,
            ins=[blk_in[:].opt()],
            outs=[h_full[bbase[k]:bbase[k] + n, :].opt()])

    with tile.TileContext(nc) as tc:
      with tc.tile_pool(name="const", bufs=1) as constp:
        # resident constants
        iota8_t = constp.tile([128, 8, 128], F16)
        ident_t = constp.tile([128, 128], F16)
        idx_t = constp.tile([128, NIDX // 16], I16)
        dl_t = constp.tile([128, NCH], F16)
        mult_t = constp.tile([128, T], F16)
        W2e_t = constp.tile([128, 4, D_OUT + 2], F16)
        shift1_t = constp.tile([128, 1], F32)
        shift2_t = constp.tile([128, 1], F32)
        nc.vector.memset(shift1_t[:], -SHIFT1)
        nc.vector.memset(shift2_t[:], -SHIFT2)

        # ---------------- GEMM1: h||al = xT.T @ W1e ----------------
        with (
            tc.tile_pool(name="gemm1", bufs=1) as g1p,
            tc.tile_pool(name="g1ps", bufs=3, space="PSUM") as g1ps,
            tc.tile_pool(name="rowio", bufs=4) as rowp,
        ):
            xT_t = g1p.tile([128, KIN, S], F16)
            W1e_t = g1p.tile([128, KIN, D_H + 2 * H1], F16)
            xT_v = xT_d[:].rearrange("(k p) n -> p k n", p=128)
            W1e_v = W1e_d[:].rearrange("(k p) n -> p k n", p=128)
            for k in range(KIN):
                nc.sync.dma_start(W1e_t[:, k, :], W1e_v[:, k, :])
                nc.sync.dma_start(xT_t[:, k, :], xT_v[:, k, :])
            # edge-phase constants load behind GEMM1's operands
            nc.sync.dma_start(iota8_t[:], iota_d[:])
            nc.sync.dma_start(ident_t[:], id_d[:])
            nc.sync.dma_start(idx_t[:], idx_d[:])
            nc.sync.dma_start(dl_t[:], dl_d[:])
            nc.sync.dma_start(mult_t[:], mult_d[:])
            nc.sync.dma_start(W2e_t[:],
                              W2e_d[:].rearrange("(k p) n -> p k n", p=128))
            for t in range(T):
                ph = g1ps.tile([128, D_H], F32, tag="g1h")
                pa = g1ps.tile([128, 2 * H1], F32, tag="g1a")
                for k in range(KIN):
                    lw = xT_t[:, k, t * 128:(t + 1) * 128]
                    nc.tensor.matmul(ph[:], lw, W1e_t[:, k, 0:D_H],
                                     start=(k == 0), stop=(k == KIN - 1))
                    nc.tensor.matmul(pa[:], lw, W1e_t[:, k, D_H:],
                                     start=(k == 0), stop=(k == KIN - 1))
                row = rowp.tile([128, ROW1], F16, tag="r1")
                ald = rowp.tile([128, H1], F16, tag="r1a")
                nc.scalar.copy(row[:, 0:H16_1], ph[:, 0:H16_1])
                nc.scalar.copy(row[:, 496:504].bitcast(F8), ph[:, H16_1:D_H])
                nc.scalar.copy(row[:, 504:512], pa[:, 0:H1])
                nc.scalar.copy(ald[:], pa[:, H1:])
                nc.sync.dma_start(loc1(t), row[:])
                nc.sync.dma_start(ald_loc[t * 128:(t + 1) * 128, :], ald[:])
                if cores > 1 and t in blk_last:
                    k = blk_last[t]
                    emit_ag(ag1b[k], h1_full, k)

        # ---------------- shared edge phase ----------------
        def edge_phase(h_full, loc_ap, ald_side, gw, rstride, h16w, heads,
                       chans, shift_t, append_p, out_cb, after_tile=None):
            D = heads * chans
            alo = h16w + (D - h16w) // 2       # f16 col of al_src in row
            vw = D + (heads if append_p else 0)
            lw = alo + heads if ald_side else alo + 2 * heads
            with (
                tc.tile_pool(name="gath", bufs=3) as gp,
                tc.tile_pool(name="ohTp", bufs=2) as ohTp,
                tc.tile_pool(name="ohp", bufs=2) as ohp,
                tc.tile_pool(name="vp", bufs=2) as vp,
                tc.tile_pool(name="ework", bufs=4) as wp,
                tc.tile_pool(name="eps", bufs=2, space="PSUM") as pp,
                tc.tile_pool(name="epsB", bufs=2, space="PSUM") as ppB,
                tc.tile_pool(name="epald", bufs=2, space="PSUM") as ppald,
                tc.tile_pool(name="epost", bufs=1, space="PSUM") as pp2,
                tc.tile_pool(name="outw", bufs=3) as op,
            ):
                for t in range(T):
                    g = gp.tile([128, k_ch, gw], F16, tag="g")
                    for c0 in range(0, k_ch, 8):
                        nch = min(8, k_ch - c0)
                        # dma_gather ucode breaks above 1024 idxs per call
                        nc.gpsimd.dma_gather(
                            g[:, c0:c0 + nch, :], h_full[:, 0:gw],
                            idx_t[:, (t * k_ch + c0) * 8:
                                  (t * k_ch + c0 + nch) * 8],
                            nch * 128, nch * 128, gw,
                            elem_step=rstride)
                    ohT_t = ohTp.tile([128, k_ch, 128], I8, tag="ohT")
                    nc.sync.dma_start(
                        ohT_t[:], ohT_d[:, t * k_ch:(t + 1) * k_ch, :])
                    # local rows: h + al_src (+ al_dst) for this tile's nodes
                    lrow = wp.tile([128, lw], F16, tag="lrow")
                    nc.sync.dma_start(lrow[:], loc_ap(t)[:, 0:lw])
                    if ald_side:
                        aldl = wp.tile([128, heads], F16, tag="aldl")
                        nc.sync.dma_start(
                            aldl[:], ald_side[t * 128:(t + 1) * 128, :])
                        ald_ap = aldl[:]
                    else:
                        ald_ap = lrow[:, alo + heads:alo + 2 * heads]
                    # al_dst in two fp8 terms (hi + residual) so the fp8
                    # one-hot matmul reconstructs it to ~0.4%
                    ald8 = wp.tile([128, heads], F8, tag="ald8")
                    nc.vector.tensor_copy(ald8[:], ald_ap)
                    aldr = wp.tile([128, heads], F16, tag="aldr")
                    nc.vector.tensor_tensor(aldr[:], ald_ap, ald8[:],
                                            op=AOT.subtract)
                    ald8r = wp.tile([128, heads], F8, tag="ald8r")
                    nc.vector.tensor_copy(ald8r[:], aldr[:])
                    psA = pp.tile([128, vw], F32, tag="eA")
                    psB = None if append_p else ppB.tile([128, heads], F32,
                                                         tag="eB")
                    for gi in range(NG):
                        c0 = gi * 8
                        ng = min(8, k_ch - c0)
                        ch0 = t * k_ch + c0
                        # batched one-hot build: oh[e, j, d] = (dl[e,j]==d)
                        oh = ohp.tile([128, 8, 128], F16, tag="oh")
                        nc.vector.tensor_tensor(
                            oh[:, 0:ng, :], iota8_t[:, 0:ng, :],
                            dl_t[:, ch0:ch0 + ng].to_broadcast([128, ng, 128]),
                            op=AOT.is_equal)
                        # al_dst per edge via fp8 transposed-one-hot matmuls
                        psald = ppald.tile([128, 8, heads], F32, tag="ald")
                        for j in range(ng):
                            nc.tensor.matmul(
                                psald[:, j, :],
                                ohT_t[:, c0 + j, :].bitcast(F8), ald8[:],
                                start=True, stop=False)
                            nc.tensor.matmul(
                                psald[:, j, :],
                                ohT_t[:, c0 + j, :].bitcast(F8), ald8r[:],
                                start=False, stop=True)
                        z = wp.tile([128, 8, heads], F32, tag="z")
                        nc.vector.tensor_tensor(
                            z[:, 0:ng, :],
                            g[:, c0:c0 + ng, alo:alo + heads],
                            psald[:, 0:ng, :], op=AOT.add)
                        nc.vector.scalar_tensor_tensor(
                            z[:, 0:ng, :], z[:, 0:ng, :], NEG, z[:, 0:ng, :],
                            op0=AOT.mult, op1=AOT.max)
                        V = vp.tile([128, 8, vw], F16, tag="v")
                        if append_p:
                            p16 = V[:, :, D:]
                        else:
                            pt = wp.tile([128, 8, heads], F16, tag="p",
                                         name="pt")
                            p16 = pt[:]
                        nc.scalar.activation(p16[:, 0:ng, :], z[:, 0:ng, :],
                                             AFT.Exp, bias=shift_t[:])
                        # V = p * h, in f16/fp8 pieces (head-uniform splits)
                        hs = (heads - 1) * chans
                        if hs > 0:
                            nc.vector.tensor_tensor(
                                V[:, 0:ng, 0:hs].rearrange(
                                    "p j (h c) -> p j h c", h=heads - 1),
                                g[:, c0:c0 + ng, 0:hs].rearrange(
                                    "p j (h c) -> p j h c", h=heads - 1),
                                p16[:, 0:ng, 0:heads - 1].to_broadcast(
                                    [128, ng, heads - 1, chans]),
                                op=AOT.mult)
                        nc.vector.tensor_tensor(
                            V[:, 0:ng, hs:h16w],
                            g[:, c0:c0 + ng, hs:h16w],
                            p16[:, 0:ng, heads - 1:heads].to_broadcast(
                                [128, ng, h16w - hs]),
                            op=AOT.mult)
                        if h16w < D:
                            nc.vector.tensor_tensor(
                                V[:, 0:ng, h16w:D],
                                g[:, c0:c0 + ng, h16w:alo].bitcast(F8),
                                p16[:, 0:ng, heads - 1:heads].to_broadcast(
                                    [128, ng, D - h16w]),
                                op=AOT.mult)
                        for j in range(ng):
                            st = (c0 + j == 0)
                            sp = (c0 + j == k_ch - 1)
                            nc.tensor.matmul(psA[:], oh[:, j, :], V[:, j, :],
                                             start=st, stop=sp)
                            if not append_p:
                                nc.tensor.matmul(psB[:], oh[:, j, :],
                                                 p16[:, j, :], start=st,
                                                 stop=sp)
                    # self-loop contribution from local rows
                    zs = wp.tile([128, heads], F32, tag="zs")
                    nc.vector.tensor_tensor(
                        zs[:], lrow[:, alo:alo + heads], ald_ap, op=AOT.add)
                    nc.vector.scalar_tensor_tensor(
                        zs[:], zs[:], NEG, zs[:], op0=AOT.mult, op1=AOT.max)
                    ps16 = wp.tile([128, heads], F16, tag="ps16")
                    nc.scalar.activation(ps16[:], zs[:], AFT.Exp,
                                         bias=shift_t[:])
                    # scale by multiplicity of data self-edges (+1 self-loop)
                    psm = wp.tile([128, heads], F16, tag="psm")
                    nc.vector.tensor_tensor(
                        psm[:], ps16[:],
                        mult_t[:, t:t + 1].to_broadcast([128, heads]),
                        op=AOT.mult)
                    sV = wp.tile([128, D], F16, tag="sv")
                    hs = (heads - 1) * chans
                    if hs > 0:
                        nc.vector.tensor_tensor(
                            sV[:, 0:hs].rearrange("p (h c) -> p h c",
                                                  h=heads - 1),
                            lrow[:, 0:hs].rearrange("p (h c) -> p h c",
                                                    h=heads - 1),
                            psm[:, 0:heads - 1].to_broadcast(
                                [128, heads - 1, chans]),
                            op=AOT.mult)
                    nc.vector.tensor_tensor(
                        sV[:, hs:h16w], lrow[:, hs:h16w],
                        psm[:, heads - 1:heads].to_broadcast([128, h16w - hs]),
                        op=AOT.mult)
                    if h16w < D:
                        nc.vector.tensor_tensor(
                            sV[:, h16w:D], lrow[:, h16w:alo].bitcast(F8),
                            psm[:, heads - 1:heads].to_broadcast(
                                [128, D - h16w]),
                            op=AOT.mult)
                    pB = psA[:, D:] if append_p else psB[:]
                    out_cb(t, psA, pB, psm, sV, wp, op, pp2)
                    if after_tile is not None:
                        after_tile(t)

        # ---------------- L1 consumer: normalize, ELU, GEMM2, row2 ----------
        def l1_consume(t, psA, psB, psm, sV, wp, op, pp2):
            s_sb = wp.tile([128, H1], F32, tag="s1")
            rinv = wp.tile([128, H1], F32, tag="ri1")
            nc.vector.scalar_tensor_tensor(
                s_sb[:], psB, EPS, psm[:], op0=AOT.add, op1=AOT.add)
            nc.vector.reciprocal(rinv[:], s_sb[:])
            num = wp.tile([128, D_H], F16, tag="num")
            nc.vector.tensor_tensor(num[:], psA[:, 0:D_H], sV[:], op=AOT.add)
            h1n = wp.tile([128, D_H], F16, tag="h1n")
            for h in range(H1):
                nc.scalar.activation(
                    h1n[:, h * C1:(h + 1) * C1], num[:, h * C1:(h + 1) * C1],
                    AFT.Copy, scale=rinv[:, h:h + 1])
            r1 = wp.tile([128, D_H], F16, tag="relu")
            m1 = wp.tile([128, D_H], F16, tag="mrelu")
            e1 = wp.tile([128, D_H], F16, tag="eneg")
            h1e = wp.tile([128, D_H], F16, tag="h1e")
            nc.scalar.activation(r1[:], h1n[:], AFT.Relu)
            nc.scalar.activation(m1[:], h1n[:], AFT.Relu, scale=-1.0)
            nc.scalar.activation(e1[:], m1[:], AFT.Exp, scale=-1.0)
            nc.vector.scalar_tensor_tensor(
                h1e[:], e1[:], -1.0, r1[:], op0=AOT.add, op1=AOT.add)
            psT = pp2.tile([128, 4, 128], F16, tag="psT")
            h1eT = wp.tile([128, 4, 128], F16, tag="h1eT")
            for k in range(4):
                nc.tensor.transpose(
                    psT[:, k, :], h1e[:, k * 128:(k + 1) * 128], ident_t[:])
                nc.scalar.copy(h1eT[:, k, :], psT[:, k, :])
            ps2 = pp2.tile([128, D_OUT + 2], F32, tag="ps2")
            for k in range(4):
                nc.tensor.matmul(ps2[:], h1eT[:, k, :], W2e_t[:, k, :],
                                 start=(k == 0), stop=(k == 3))
            row2 = op.tile([128, D_OUT + 2], F16, tag="r2")
            nc.scalar.copy(row2[:, 0:D_OUT], ps2[:, 0:D_OUT])
            nc.scalar.copy(row2[:, D_OUT:D_OUT + 2], ps2[:, D_OUT:D_OUT + 2])
            nc.sync.dma_start(loc2(t)[:, 0:D_OUT + 2], row2[:])

        def l1_after(t):
            if cores > 1 and t in blk_last:
                k = blk_last[t]
                emit_ag(ag2b[k], h2_full, k)

        edge_phase(h1_full, loc1, ald_loc, G1W, ROW1, H16_1, H1, C1,
                   shift1_t, False, l1_consume, l1_after)

        # ---------------- L2 consumer: normalize -> output ----------
        def l2_consume(t, psA, psB, psm, sV, wp, op, pp2):
            s_sb = wp.tile([128, 1], F32, tag="s2")
            rinv = wp.tile([128, 1], F32, tag="ri2")
            nc.vector.scalar_tensor_tensor(
                s_sb[:], psB, EPS, psm[:], op0=AOT.add, op1=AOT.add)
            nc.vector.reciprocal(rinv[:], s_sb[:])
            num = wp.tile([128, D_OUT], F16, tag="num2")
            nc.vector.tensor_tensor(num[:], psA[:, 0:D_OUT], sV[:],
                                    op=AOT.add)
            o = op.tile([128, D_OUT], F32, tag="of")
            nc.scalar.activation(o[:], num[:], AFT.Copy, scale=rinv[:])
            nc.sync.dma_start(out_d[t * 128:(t + 1) * 128, :], o[:])

        edge_phase(h2_full, loc2, None, G2W, ROW2, H16_2, 1, D_OUT,
                   shift2_t, True, l2_consume)

    nc.compile()
    return nc


# ---------------------------------------------------------------- entry point

def make_in_maps(cfg, host):
    return [{
        "xT": host["xT"][c], "W1e": host["W1e"], "W2e": host["W2e"],
        "idx": host["idx"][c], "dloc": host["dloc"][c], "ohT": host["ohT"][c],
        "mult": host["mult"][c], "iota8": host["iota8"],
        "ident": host["ident"],
    } for c in range(cfg["cores"])]


def kernel(x, edge_index, W1, a1_src, a1_dst, b1, W2, a2_src, a2_dst, b2,
           cores=8, runner=None):
    x = np.asarray(x)
    edge_index = np.asarray(edge_index)
    assert np.allclose(np.asarray(b1), 0.0), "nonzero b1 unsupported"
    cfg, host, post = prep(x, edge_index, W1, a1_src, a1_dst,
                           W2, a2_src, a2_dst, cores)
    nc = build(cfg)
    in_maps = make_in_maps(cfg, host)
    if runner is None:
        from concourse.bass_utils import run_bass_kernel_spmd
        res = run_bass_kernel_spmd(nc, in_maps, core_ids=list(range(cores)),
                                   trace=os.environ.get("GAT_TRACE", "") == "1")
        outs = [r["out"] for r in res.results]
        kernel.last_exec_ns = res.exec_time_ns
        kernel.last_result = res
    else:
        outs = runner(nc, in_maps)
    slots = np.concatenate(outs, 0)          # [cores*S, D_OUT]
    full = slots[post["slot"]]               # back to node order
    return (full + np.asarray(b2)[None, :]).astype(np.float32)


kernel.last_exec_ns = None
